# revision 6
# baseline (speedup 1.0000x reference)
"""Trainium2 Bass kernel for nn_MultiHeadNetwork (moe_routing) — v4.

On top of kernel3 (bf16, batched weight DMA, resident head weights):
  * Packing heuristics that reliably reach S=4 head slots per 256-col
    window (sorted order gives S=5): several greedy sequencers are tried,
    best S wins.  Head matmuls drop 320 -> 256 per core.
  * One-hot fold: the trailing 50 one-hot features of x contribute
    W0[2048+tid] per row — a per-column bias.  Host gathers that map and the
    device adds it to the layer-0 PSUM on the vector engine, dropping the
    17th k-chunk of layer 0 (16 matmuls/core).

v5: tiered head-slot widths.  Within each 256-col window the tasks are
ordered by descending row count; slot s's task region then provably lies in
cols [64*s, 256), so slot matmuls can use N = 256-64*s (256/192/128/64)
instead of 4x256 — a 37.5% cut in head streamed columns (PE time here is
~21ns + N*0.527ns per matmul; the clock sits at ~1.9GHz under sustained
load, weight loads are already hidden).

v6: data-derived slot bounds.  Every slot (including rank 0) is a masked
copy, so slot rank s only needs to cover [min_w start, max_w end) over the
real windows — measured ~450 cols total per window instead of 640.  The
program is cached per (S, bounds).

v7: prefix-uniform "tier" packing (rank-0 = 16 largest tasks, rank-1 pairs
large+small, rank-2 targets uniform prefix, leftovers split as small
fillers); packings scored by modeled PE cost (~21ns + N*0.527ns per MM).

v8: single-shot pipeline-fill trims: layer-0 quarter-0 loads weights per
k-chunk so the first matmul issues after ~0.4us of DMA instead of ~3-8us;
mask DMA deferred to just before the head; one-hot maps loaded per quarter.
Steady-state is unchanged (PE-bound); this only shortens the program span
a single execution pays.
"""

import numpy as np
import ml_dtypes
from contextlib import ExitStack

import concourse.bacc as bacc
import concourse.mybir as mybir
from concourse.tile import TileContext
from concourse import bass_utils

BATCH = 4096
FEAT = 2048
NUM_TASKS = 50
WIDTH = 2048
HEAD_DIM = 256
NCORES = 8
BPC = BATCH // NCORES          # 512 rows per core
NWIN = 2                       # head column windows per core
WINC = BPC // NWIN             # 256 columns per window
NWTOT = NCORES * NWIN          # 16 windows
KC = WIDTH // 128              # 16 (also layer-0 k-chunks after fold)
WC = WIDTH // 128              # 16
QW = 4                         # w-chunks per trunk quarter (4 PSUM banks)
NQ = WC // QW                  # 4 quarters per layer
MH = HEAD_DIM // 128           # 2 head-dim halves

F32 = mybir.dt.float32
BF16 = mybir.dt.bfloat16
U8 = mybir.dt.uint8
NPBF16 = ml_dtypes.bfloat16

_PROG_CACHE: dict = {}


_LAST_BOUNDS = None


def _build(S: int, repeat: int = 1, bounds=None):
    """Build + compile the SPMD Tile program.

    bounds: per window-index, a tuple of per-rank (lo, hi) column ranges —
    ((w0_rank0, w0_rank1, ...), (w1_rank0, ...)).  Slot counts may differ
    between the two windows (the program is per-core uniform, not
    per-window uniform).  None falls back to the last bounds prepare()
    derived, else S full-width ranks per window.  The legacy scalar S is
    only used for that fallback."""
    global _LAST_BOUNDS
    if bounds is None:
        bounds = _LAST_BOUNDS or tuple(
            tuple((0, WINC) for _ in range(S)) for _ in range(NWIN)
        )
    S_w = [len(b) for b in bounds]
    slot_base = [0, S_w[0]]
    nslot = sum(S_w)
    nc = bacc.Bacc("TRN2", target_bir_lowering=False, debug=False)
    xT = nc.dram_tensor("xT", [KC, 128, BPC], BF16, kind="ExternalInput").ap()
    w0 = nc.dram_tensor("w0p", [NQ, 128, KC * 512], BF16, kind="ExternalInput").ap()
    w1 = nc.dram_tensor("w1p", [NQ, 128, KC * 512], BF16, kind="ExternalInput").ap()
    w2 = nc.dram_tensor("w2p", [NQ, 128, KC * 512], BF16, kind="ExternalInput").ap()
    ohm = nc.dram_tensor("ohm", [WC, 128, BPC], BF16, kind="ExternalInput").ap()
    bia = nc.dram_tensor("bias", [128, 3 * WC], F32, kind="ExternalInput").ap()
    hws = nc.dram_tensor("hws", [nslot, 128, KC * HEAD_DIM], BF16, kind="ExternalInput").ap()
    msk = nc.dram_tensor("msk", [128, nslot * WINC], U8, kind="ExternalInput").ap()
    out = nc.dram_tensor("outT", [MH, 128, BPC], F32, kind="ExternalOutput").ap()

    with TileContext(nc) as tc, ExitStack() as ctx:
        actA = ctx.enter_context(tc.tile_pool(name="actA", bufs=KC))
        actB = ctx.enter_context(tc.tile_pool(name="actB", bufs=KC))
        wp = ctx.enter_context(tc.tile_pool(name="wp", bufs=3))
        wq0 = ctx.enter_context(tc.tile_pool(name="wq0", bufs=KC))
        ohp = ctx.enter_context(tc.tile_pool(name="ohp", bufs=WC))
        cons = ctx.enter_context(tc.tile_pool(name="cons", bufs=1))
        hwp = ctx.enter_context(tc.tile_pool(name="hwp", bufs=nslot))
        op = ctx.enter_context(tc.tile_pool(name="op", bufs=MH))
        psp = ctx.enter_context(tc.tile_pool(name="psp", bufs=8, space="PSUM"))

        if repeat > 1:
            ctx.enter_context(tc.For_i(0, repeat, 1))

        bt = cons.tile([128, 3 * WC], F32, tag="bt")
        mt = cons.tile([128, nslot * WINC], U8, tag="mt")

        xt = [None] * KC
        oht = [None] * WC

        def trunk_layer(src, wdram, li, pool, tag, load_x=False):
            outs = [None] * WC
            nk = KC
            nkA = nk // 2
            for q in range(NQ):
                fast_start = load_x and q == 0
                if li == 0:
                    # one-hot bias maps for this quarter (needed at its end)
                    for w in range(QW):
                        wc_i = q * QW + w
                        o = ohp.tile([128, BPC], BF16, tag="ohp", name=f"oh{wc_i}")
                        nc.sync.dma_start(o[:], ohm[wc_i])
                        oht[wc_i] = o
                if fast_start:
                    # per-k weight loads so the first matmul starts after one
                    # small DMA instead of a whole half-quarter batch
                    wts = []
                    for k in range(nk):
                        t = actA.tile([128, BPC], BF16, tag="actA", name=f"xt{k}")
                        nc.sync.dma_start(t[:], xT[k])
                        src[k] = t
                        wk = wq0.tile([128, 512], BF16, tag="wq0", name=f"wq0k{k}")
                        nc.sync.dma_start(
                            wk[:], wdram[q, :, k * 512:(k + 1) * 512]
                        )
                        wts.append(wk)
                        if k == 1:
                            nc.sync.dma_start(bt[:], bia)
                else:
                    wtA = wp.tile([128, nkA * 512], BF16, tag="wp", name=f"wA{li}q{q}")
                    nc.sync.dma_start(wtA[:], wdram[q, :, : nkA * 512])
                    wtB = wp.tile([128, (nk - nkA) * 512], BF16, tag="wp", name=f"wB{li}q{q}")
                    nc.sync.dma_start(wtB[:], wdram[q, :, nkA * 512:])
                pss = [
                    psp.tile([128, BPC], F32, tag="ps", name=f"psL{li}q{q}w{w}")
                    for w in range(QW)
                ]
                for k in range(nk):
                    if fast_start:
                        wt, kk = wts[k], 0
                    else:
                        wt, kk = (wtA, k) if k < nkA else (wtB, k - nkA)
                    for w in range(QW):
                        nc.tensor.matmul(
                            pss[w][:],
                            wt[:, kk * 512 + w * 128: kk * 512 + (w + 1) * 128],
                            src[k][:],
                            start=(k == 0),
                            stop=(k == nk - 1),
                        )
                for w in range(QW):
                    wc_i = q * QW + w
                    if li == 0:
                        nc.vector.tensor_add(pss[w][:], pss[w][:], oht[wc_i][:])
                    h = pool.tile([128, BPC], BF16, tag=tag, name=f"h{li}_{wc_i}")
                    nc.scalar.activation(
                        h[:], pss[w][:], mybir.ActivationFunctionType.Relu,
                        bias=bt[:, li * WC + wc_i: li * WC + wc_i + 1],
                    )
                    outs[wc_i] = h
            return outs

        h1 = trunk_layer(xt, w0, 0, actB, "actB", load_x=True)
        h2 = trunk_layer(h1, w1, 1, actA, "actA")

        # issue head-weight + mask DMAs before layer 2 so they overlap it
        nc.sync.dma_start(mt[:], msk)
        hwt = []
        for sl in range(nslot):
            hw = hwp.tile([128, KC * HEAD_DIM], BF16, tag="hwp", name=f"hw{sl}")
            nc.sync.dma_start(hw[:], hws[sl])
            hwt.append(hw)

        h3 = trunk_layer(h2, w2, 2, actB, "actB")

        # every slot is a masked copy over its data-derived column range;
        # each window column belongs to exactly one task, so coverage is exact
        om = [op.tile([128, BPC], F32, tag="op", name=f"om{m}") for m in range(MH)]
        for win in range(NWIN):
            cols = slice(win * WINC, (win + 1) * WINC)
            for s in range(S_w[win]):
                sl = slot_base[win] + s
                hw = hwt[sl]
                lo, hi = bounds[win][s]
                sw = hi - lo
                scols = slice(win * WINC + lo, win * WINC + hi)
                for m in range(MH):
                    ps = psp.tile([128, sw], F32, tag="ps", name=f"psH{sl}m{m}")
                    for k in range(KC):
                        nc.tensor.matmul(
                            ps[:],
                            hw[:, k * HEAD_DIM + m * 128: k * HEAD_DIM + (m + 1) * 128],
                            h3[k][:, scols],
                            start=(k == 0),
                            stop=(k == KC - 1),
                        )
                    nc.vector.copy_predicated(
                        om[m][:, scols],
                        mt[:, sl * WINC + lo: sl * WINC + hi],
                        ps[:],
                    )
            for m in range(MH):
                nc.sync.dma_start(out[m][:, cols], om[m][:, cols])

    nc.compile()
    return nc


def _pack_w(W):
    # [NQ, 128, KC*512]; [q, p, k*512 + j] = W[k*128 + p, q*512 + j]
    return np.ascontiguousarray(
        W.reshape(KC, 128, NQ, 512).transpose(2, 1, 0, 3).reshape(NQ, 128, KC * 512)
    )


def _seq_pack(sizes, variant):
    """Sequence (task, nrows) chunks so each 256-row window spans few tasks.

    Returns list of windows, each a list of (task, nrows)."""
    remaining = {t: s for t, s in enumerate(sizes) if s > 0}
    windows = []
    carry = None  # (task, nrows left)
    for w in range(NWTOT):
        cap = WINC
        cur = []
        if carry is not None:
            t, n = carry
            take = min(cap, n)
            cur.append((t, take))
            cap -= take
            carry = (t, n - take) if n - take else None
        # place whole tasks
        while cap > 0 and carry is None:
            avail = [t for t, s in remaining.items() if s <= cap]
            exact = [t for t in avail if remaining[t] == cap]
            if exact:
                t = exact[0]
            elif avail and len(cur) < 3:
                if variant == "big":
                    t = max(avail, key=lambda t: remaining[t])
                else:
                    t = min(avail, key=lambda t: remaining[t])
            else:
                t = None
            if t is not None:
                cur.append((t, remaining.pop(t)))
                cap -= cur[-1][1]
            else:
                break
        if cap > 0:
            # split-fill from the largest remaining task
            t = max(remaining, key=lambda t: remaining[t])
            n = remaining.pop(t)
            take = min(cap, n)
            cur.append((t, take))
            cap -= take
            if n - take:
                carry = (t, n - take)
        assert cap == 0, (w, cap)
        windows.append(cur)
    assert carry is None and not remaining
    return windows


def _tier_pack(sizes):
    """Prefix-uniform packing: rank-0 = the 16 largest tasks, rank-1 pairs
    large with small to equalize A+B, rank-2 targets a uniform A+B+C, and
    the leftover tasks are split as small rank-3 fillers.  Tight per-rank
    prefix ranges mean narrow data-derived slot bounds (small head matmuls).

    Returns windows as [(task, nrows)] or raises AssertionError."""
    tasks = [t for t in range(NUM_TASKS) if sizes[t] > 0]
    assert len(tasks) >= 3 * NWTOT + 1
    desc = sorted(tasks, key=lambda t: -sizes[t])
    A = desc[:NWTOT]                          # largest, one per window
    B = desc[NWTOT:2 * NWTOT]
    rest = desc[2 * NWTOT:]
    # pair largest A with smallest B to equalize A+B
    A = sorted(A, key=lambda t: -sizes[t])
    B = sorted(B, key=lambda t: sizes[t])
    win = [[(a, int(sizes[a])), (b, int(sizes[b]))] for a, b in zip(A, B)]
    # choose C per window targeting uniform A+B+C with fill in [6, 48]
    rest_pool = set(rest)
    fill_need = []
    for w in range(NWTOT):
        ab = win[w][0][1] + win[w][1][1]
        # pick C leaving fill between 6 and 48
        cands = [t for t in rest_pool if 6 <= WINC - ab - sizes[t] <= 48]
        assert cands, (w, ab)
        c = min(cands, key=lambda t: abs(WINC - ab - sizes[t] - 16))
        rest_pool.remove(c)
        win[w].append((c, int(sizes[c])))
        fill_need.append(WINC - ab - int(sizes[c]))
    # leftover tasks fill the gaps as split rank-3 chunks
    assert sum(fill_need) == sum(int(sizes[t]) for t in rest_pool)
    fillers = sorted(rest_pool, key=lambda t: -sizes[t])
    fi, frem = 0, int(sizes[fillers[0]]) if fillers else 0
    for w in sorted(range(NWTOT), key=lambda w: -fill_need[w]):
        need = fill_need[w]
        while need > 0:
            assert fi < len(fillers)
            t = fillers[fi]
            take = min(need, frem)
            assert take > 0
            # a filler task must not repeat within one window
            assert all(tt != t for tt, _ in win[w])
            win[w].append((t, take))
            need -= take
            frem -= take
            if frem == 0:
                fi += 1
                frem = int(sizes[fillers[fi]]) if fi < len(fillers) else 0
    assert all(sum(n for _, n in w) == WINC for w in win)
    assert max(len(w) for w in win) <= 5
    return win


def _balance(tid):
    """Minimize S (max distinct tasks per 256-row window), then tightness of
    per-rank prefix ranges.  Returns (order, win_tasks) like kernel3."""
    sizes = np.bincount(tid, minlength=NUM_TASKS)

    def score(ws):
        """Modeled head PE cost in ns (matmul ~ 21ns + N*0.527ns)."""
        s_max = max(len(w) for w in ws)
        lo = [WINC] * s_max
        hi = [0] * s_max
        for w in ws:
            pre = 0
            for s, (t, n) in enumerate(sorted(w, key=lambda tn: -tn[1])):
                lo[s] = min(lo[s], pre)
                hi[s] = max(hi[s], pre + n)
                pre += n
        width = sum(h - l for l, h in zip(lo, hi))
        return NWIN * 2 * KC * (s_max * 21.0 + width * 0.527)

    best = None
    for maker in (
        lambda: _tier_pack(sizes),
        lambda: _seq_pack(sizes, "small"),
        lambda: _seq_pack(sizes, "big"),
    ):
        try:
            ws = maker()
        except (AssertionError, IndexError):
            continue
        sc = score(ws)
        if best is None or sc < best[0]:
            best = (sc, ws)
    # fallback: plain sorted order
    t_sorted = np.sort(tid)
    ws_sorted = []
    for w in range(NWTOT):
        ch = t_sorted[w * WINC:(w + 1) * WINC]
        tl, cnts = np.unique(ch, return_counts=True)
        # preserve appearance order
        seen = list(dict.fromkeys(ch.tolist()))
        ws_sorted.append([(t, int((ch == t).sum())) for t in seen])
    sc_sorted = score(ws_sorted)
    if best is None or sc_sorted < best[0]:
        best = (sc_sorted, ws_sorted)

    _, windows = best
    rows_by_task = {t: list(np.nonzero(tid == t)[0]) for t in range(NUM_TASKS)}
    ptr = {t: 0 for t in range(NUM_TASKS)}
    order = []
    win_tasks = []
    for w in range(NWTOT):
        tl = []
        # desc by in-window size: slot s's region then starts at col >= 64*s
        for t, n in sorted(windows[w], key=lambda tn: -tn[1]):
            p = ptr[t]
            order.extend(rows_by_task[t][p:p + n])
            ptr[t] = p + n
            tl.append(t)
        win_tasks.append(tl)
    return np.asarray(order), win_tasks


def prepare(x, W0, b0, W1, b1, W2, b2, head_W, head_b):
    """Host-side sharding. Returns (in_maps, order, sorted_task_ids, S)."""
    x = np.asarray(x, np.float32)
    W0 = np.asarray(W0, np.float32)
    W1 = np.asarray(W1, np.float32)
    W2 = np.asarray(W2, np.float32)
    b0 = np.asarray(b0, np.float32)
    b1 = np.asarray(b1, np.float32)
    b2 = np.asarray(b2, np.float32)
    head_W = np.asarray(head_W, np.float32)

    tid = np.argmax(x[:, -NUM_TASKS:], axis=1)
    order, win_tasks_flat = _balance(tid)
    x_s = x[order]
    t_s = tid[order]

    win_tasks = [
        [win_tasks_flat[c * NWIN + w] for w in range(NWIN)] for c in range(NCORES)
    ]
    S = max(len(tl) for per in win_tasks for tl in per)

    w0p = _pack_w(W0[:FEAT].astype(NPBF16))
    w1p = _pack_w(W1.astype(NPBF16))
    w2p = _pack_w(W2.astype(NPBF16))
    bias = np.zeros((128, 3 * WC), np.float32)
    for li, b in enumerate((b0, b1, b2)):
        bias[:, li * WC:(li + 1) * WC] = b.reshape(WC, 128).T

    # one-hot fold: per-row bias row W0[FEAT + tid], laid out like the
    # layer-0 psum [w-chunk, 128, col]
    W0oh16 = W0[FEAT:].astype(NPBF16)          # [NUM_TASKS, WIDTH]

    head_W16 = head_W.astype(NPBF16)
    hw_pack = np.ascontiguousarray(
        head_W16.reshape(NUM_TASKS, KC, 128, HEAD_DIM)
        .transpose(0, 2, 1, 3)
        .reshape(NUM_TASKS, 128, KC * HEAD_DIM)
    )

    # per-window rank spans (lo, hi) from actual positions
    def win_ranks(t_s_loc, wtf):
        ranks = []
        for w in range(NWTOT):
            ch = t_s_loc[w * WINC:(w + 1) * WINC]
            rr = []
            for t in wtf[w]:
                pos = np.nonzero(ch == t)[0]
                rr.append((int(pos.min()), int(pos.max()) + 1))
            ranks.append(rr)
        return ranks

    ranks = win_ranks(t_s, win_tasks_flat)

    # choose, per core, which 256-row block is window 0 vs 1 so the
    # per-window-index rank bounds are tightest (program is emitted per
    # window index, so slot count + bounds can differ between the two)
    def combo_cost(swaps):
        cost = 0.0
        for idx in range(NWIN):
            wins = [2 * c + (idx ^ swaps[c]) for c in range(NCORES)]
            smax = max(len(ranks[w]) for w in wins)
            for r in range(smax):
                spans = [ranks[w][r] for w in wins if len(ranks[w]) > r]
                lo = min(s[0] for s in spans)
                hi = max(s[1] for s in spans)
                cost += 21.0 + (hi - lo) * 0.527
        return cost

    best_sw = min(
        (tuple((m >> c) & 1 for c in range(NCORES)) for m in range(1 << NCORES)),
        key=combo_cost,
    )
    if any(best_sw):
        order = order.copy()
        for c in range(NCORES):
            if best_sw[c]:
                a = slice(c * BPC, c * BPC + WINC)
                b = slice(c * BPC + WINC, (c + 1) * BPC)
                order[a], order[b] = order[b].copy(), order[a].copy()
                w0, w1 = 2 * c, 2 * c + 1
                win_tasks_flat[w0], win_tasks_flat[w1] = (
                    win_tasks_flat[w1], win_tasks_flat[w0],
                )
        x_s = x[order]
        t_s = tid[order]
        ranks = win_ranks(t_s, win_tasks_flat)
        win_tasks = [
            [win_tasks_flat[c * NWIN + w] for w in range(NWIN)]
            for c in range(NCORES)
        ]

    # data-derived bounds per (window index, rank)
    bounds = []
    S_w = []
    for idx in range(NWIN):
        wins = [c * NWIN + idx for c in range(NCORES)]
        smax = max(len(ranks[w]) for w in wins)
        S_w.append(smax)
        bb = []
        for r in range(smax):
            spans = [ranks[w][r] for w in wins if len(ranks[w]) > r]
            bb.append((min(s[0] for s in spans), max(s[1] for s in spans)))
        bounds.append(tuple(bb))
    bounds = tuple(bounds)
    global _LAST_BOUNDS
    _LAST_BOUNDS = bounds

    slot_base = [0, S_w[0]]
    nslot = sum(S_w)
    in_maps = []
    for c in range(NCORES):
        rows = slice(c * BPC, (c + 1) * BPC)
        xs = x_s[rows]
        xTp = np.ascontiguousarray(
            xs[:, :FEAT].T.astype(NPBF16).reshape(KC, 128, BPC)
        )
        ohm_c = np.ascontiguousarray(
            W0oh16[t_s[rows]].T.reshape(WC, 128, BPC)
        )
        slot_tasks = []
        msk_c = np.zeros((128, nslot * WINC), np.uint8)
        for w in range(NWIN):
            tl = win_tasks[c][w]
            tl_p = tl + [tl[-1]] * (S_w[w] - len(tl))
            lo = c * BPC + w * WINC
            ch = t_s[lo: lo + WINC]
            for s, t in enumerate(tl_p):
                sl = slot_base[w] + s
                slot_tasks.append(t)
                if s < len(tl):
                    msk_c[:, sl * WINC:(sl + 1) * WINC] = (ch == t)[None, :].astype(np.uint8)
        hws_c = np.ascontiguousarray(hw_pack[np.asarray(slot_tasks)])
        in_maps.append({
            "xT": xTp, "ohm": ohm_c,
            "w0p": w0p, "w1p": w1p, "w2p": w2p, "bias": bias,
            "hws": hws_c, "msk": msk_c,
        })
    return in_maps, order, t_s, S


def _assemble(results, order, t_s, head_b):
    head_b = np.asarray(head_b, np.float32)
    outs = []
    for c in range(NCORES):
        oT = results[c]["outT"]                       # [MH, 128, BPC]
        outs.append(oT.reshape(HEAD_DIM, BPC).T)      # [BPC, 256]
    out_s = np.concatenate(outs, axis=0) + head_b[t_s]
    out = np.empty_like(out_s)
    out[order] = out_s
    return out.astype(np.float32)


def kernel(x, W0, b0, W1, b1, W2, b2, head_W, head_b):
    in_maps, order, t_s, S = prepare(x, W0, b0, W1, b1, W2, b2, head_W, head_b)
    key = (S, _LAST_BOUNDS)
    nc = _PROG_CACHE.get(key)
    if nc is None:
        nc = _build(S, bounds=_LAST_BOUNDS)
        _PROG_CACHE[key] = nc
    res = bass_utils.run_bass_kernel_spmd(nc, in_maps, core_ids=list(range(NCORES)))
    return _assemble(res.results, order, t_s, head_b)
